# revision 7
# baseline (speedup 1.0000x reference)
"""Masked multi-head attention (B=2, S=2048, E=1024, H=16) on 8 trn2 cores.

Sharding: tensor-parallel over heads. Each core owns 2 heads (a 128-wide
feature slice of the Q/K/V projections and of Wo's input side), computes the
full projections + causal attention + its partial of the output projection
for all tokens, and writes a full-shape partial output. The host sums the 8
partials and adds the bias terms.

Device-side layout avoids all large transposes:
  - qT/kT in [feature, token] layout straight from the projection matmul
  - scores computed transposed, sT[k, q] = kT_blk.T @ qT_blk, so the padding
    mask is a per-partition bias of the exp() activation
  - softmax without max-subtraction (scores are O(+-8) here; exp is safe in
    f32) with the denominator produced for free by appending a ones column
    to V in the o = p.T @ [V|1] matmul
  - only one 128x128 PE transpose per q-tile (o -> oT for the out-proj)

Bias handling: bk provably cancels in softmax (constant per row); bv and bo
commute with the attention average (rows of attn sum to 1) so both fold into
a host-side constant vector bv @ Wo.T + bo added during the unshard.
"""

import numpy as np

B, S, E, H, D = 2, 2048, 1024, 16, 64
NCORES = 8
FLOC = (H // NCORES) * D  # 128 features per core (2 heads)
T = B * S  # 4096 tokens
NEG = -1e30

_cache = {}


def _ensure_imports():
    try:
        import concourse.bass  # noqa: F401
    except ImportError:
        import sys

        sys.path.append("/opt/trn_rl_repo")


def _patch_tile_drain():
    """walrus CoreV2/V3 codegen accepts only one sync-wait per CTRL
    instruction; TileContext's tail drain carries one per active proc.
    Split them across single-wait nops."""
    import concourse.mybir as mybir
    import concourse.tile as tile_mod
    from concourse.vector_clock import ScopedClock

    if getattr(tile_mod.TileContext, "_drain_patched", False):
        return

    def _drain_and_barrier(self, tick_clock, wait_clock):
        nc = self.nc
        probe = nc.sync.nop(nofuse=True, hint="tail_wait_probe")
        wait_clock.add_sem_waits(
            probe.ins, ScopedClock({None: tick_clock.global_clock})
        )
        waits = list(probe.ins.sync_info.on_wait)
        del probe.ins.sync_info.on_wait[:]
        probe.ins.sync_info.on_wait.extend(waits[:1])
        for i, w in enumerate(waits[1:]):
            carrier = nc.sync.nop(nofuse=True, hint=f"tail_wait_{i}")
            if carrier.ins.sync_info is None:
                carrier.ins.sync_info = mybir.SyncInfo(on_wait=[w], on_update=[])
            else:
                carrier.ins.sync_info.on_wait.append(w)
        nc.sync.drain()
        nc.all_engine_barrier()
        assert self.sems is not None
        popped = nc._tile_sem_poison_stack.pop()
        assert popped is self._sem_poison
        nc.clear_and_free_semaphores(list(self.sems.allocated().values()))
        nc.all_engine_barrier()

    tile_mod.TileContext._drain_and_barrier = _drain_and_barrier
    tile_mod.TileContext._drain_patched = True


def _split_multi_waits(nc):
    """This walrus build accepts only one sync-wait per instruction on
    several instruction encodings (CTRL, LDWEIGHTS). Move extra waits onto
    same-engine nop carriers placed right before the instruction —
    semantically identical (engine queues are in-order)."""
    import concourse.mybir as mybir

    for bb in nc.m.functions[0].blocks:
        new_insts = []
        changed = False
        for inst in bb.instructions:
            si = getattr(inst, "sync_info", None)
            if si is not None and len(si.on_wait) > 1:
                waits = list(si.on_wait)
                del si.on_wait[:]
                si.on_wait.append(waits[-1])
                for i, w in enumerate(waits[:-1]):
                    nop = mybir.InstNoOp(
                        name=f"{inst.name}-sw{i}",
                        engine=inst.engine,
                        sync_info=mybir.SyncInfo(on_wait=[w], on_update=[]),
                        bass_nofuse=True,
                    )
                    nc.register_instruction(nop, overwrite=True)
                    new_insts.append(nop)
                changed = True
            new_insts.append(inst)
        if changed:
            bb.instructions[:] = new_insts


def _build_program():
    from contextlib import ExitStack

    import concourse.bass as bass
    import concourse.mybir as mybir
    import concourse.tile as tile
    from concourse.masks import make_identity

    f32 = mybir.dt.float32
    AF = mybir.ActivationFunctionType

    nc = bass.Bass()
    xT_d = nc.declare_dram_parameter("xT", [8, 128, T], f32, isOutput=False)
    wq_d = nc.declare_dram_parameter("wqT", [8, 128, 128], f32, isOutput=False)
    wk_d = nc.declare_dram_parameter("wkT", [8, 128, 128], f32, isOutput=False)
    wv_d = nc.declare_dram_parameter("wvT", [8, 128, 128], f32, isOutput=False)
    wo_d = nc.declare_dram_parameter("woT", [128, E], f32, isOutput=False)
    bq_d = nc.declare_dram_parameter("bq", [128, 1], f32, isOutput=False)
    mk_d = nc.declare_dram_parameter("maskb", [128, B * 16], f32, isOutput=False)
    out_d = nc.declare_dram_parameter("out", [T, E], f32, isOutput=True)

    NQT = S // 128  # 16 q/k tiles per batch

    with tile.TileContext(nc) as tc:
        with ExitStack() as ctx:
            consts = ctx.enter_context(tc.tile_pool(name="consts", bufs=1))
            xpool = ctx.enter_context(tc.tile_pool(name="xpool", bufs=2))
            qkv = ctx.enter_context(tc.tile_pool(name="qkv", bufs=2))
            ppool = ctx.enter_context(tc.tile_pool(name="ppool", bufs=3))
            spool = ctx.enter_context(tc.tile_pool(name="spool", bufs=2))
            pbig = ctx.enter_context(tc.tile_pool(name="pbig", bufs=3, space="PSUM"))
            psml = ctx.enter_context(tc.tile_pool(name="psml", bufs=3, space="PSUM"))
            pso = ctx.enter_context(tc.tile_pool(name="pso", bufs=1, space="PSUM"))

            wq_sb = consts.tile([128, 8, 128], f32)
            wk_sb = consts.tile([128, 8, 128], f32)
            wv_sb = consts.tile([128, 8, 128], f32)
            wo_sb = consts.tile([128, E], f32)
            bq_sb = consts.tile([128, 1], f32)
            mk_sb = consts.tile([128, B * 16], f32)
            nc.sync.dma_start(out=wq_sb, in_=wq_d[:].rearrange("e p f -> p e f"))
            nc.sync.dma_start(out=wk_sb, in_=wk_d[:].rearrange("e p f -> p e f"))
            nc.sync.dma_start(out=wv_sb, in_=wv_d[:].rearrange("e p f -> p e f"))
            nc.sync.dma_start(out=wo_sb, in_=wo_d[:])
            nc.sync.dma_start(out=bq_sb, in_=bq_d[:])
            nc.sync.dma_start(out=mk_sb, in_=mk_d[:])

            ident = consts.tile([128, 128], f32)
            make_identity(nc, ident)
            causal = consts.tile([128, 128], f32)
            # keep p[k, q] where k <= q within the diagonal block, else 0
            nc.gpsimd.memset(causal, 1.0)
            nc.gpsimd.affine_select(
                out=causal,
                in_=causal,
                compare_op=mybir.AluOpType.is_ge,
                fill=0.0,
                base=0,
                pattern=[[1, 128]],
                channel_multiplier=-1,
            )

            for b in range(B):
                # ---- projections for this batch: qT/kT [feat, tok], v [tok, feat|1]
                qT = qkv.tile([128, S], f32, tag="qT")
                kT = qkv.tile([128, S], f32, tag="kT")
                v = qkv.tile([128, NQT, 130], f32, tag="v")
                nc.gpsimd.memset(v[:, :, 64:65], 1.0)
                nc.gpsimd.memset(v[:, :, 129:130], 1.0)
                for tc4 in range(4):
                    g0 = b * S + tc4 * 512
                    xt = xpool.tile([128, 8, 512], f32, tag="xT")
                    nc.sync.dma_start(
                        out=xt,
                        in_=xT_d[:, :, g0 : g0 + 512].rearrange("e p t -> p e t"),
                    )
                    q_ps = pbig.tile([128, 512], f32, tag="big")
                    k_ps = pbig.tile([128, 512], f32, tag="big")
                    v_ps = pbig.tile([128, 512], f32, tag="big")
                    for e in range(8):
                        st, sp = (e == 0), (e == 7)
                        nc.tensor.matmul(
                            q_ps, lhsT=wq_sb[:, e, :], rhs=xt[:, e, :], start=st, stop=sp
                        )
                        nc.tensor.matmul(
                            k_ps, lhsT=wk_sb[:, e, :], rhs=xt[:, e, :], start=st, stop=sp
                        )
                    for tt in range(4):
                        for e in range(8):
                            nc.tensor.matmul(
                                v_ps[:, tt * 128 : (tt + 1) * 128],
                                lhsT=xt[:, e, tt * 128 : (tt + 1) * 128],
                                rhs=wv_sb[:, e, :],
                                start=(e == 0),
                                stop=(e == 7),
                            )
                    cs = slice(tc4 * 512, (tc4 + 1) * 512)
                    nc.vector.tensor_scalar_add(qT[:, cs], q_ps, bq_sb)
                    nc.vector.tensor_copy(kT[:, cs], k_ps)
                    for tt in range(4):
                        ti = tc4 * 4 + tt
                        nc.vector.tensor_copy(
                            v[:, ti, 0:64], v_ps[:, tt * 128 : tt * 128 + 64]
                        )
                        nc.vector.tensor_copy(
                            v[:, ti, 65:129], v_ps[:, tt * 128 + 64 : (tt + 1) * 128]
                        )

                # ---- causal attention + out-projection, per 128-row q tile
                for qi in range(NQT):
                    qs = slice(qi * 128, (qi + 1) * 128)
                    o_psA = pso.tile([128, 65], f32, tag="oA")
                    o_psB = pso.tile([128, 65], f32, tag="oB")
                    for kj in range(qi + 1):
                        ks = slice(kj * 128, (kj + 1) * 128)
                        sA = psml.tile([128, 128], f32, tag="sblk")
                        sB = psml.tile([128, 128], f32, tag="sblk")
                        nc.tensor.matmul(
                            sA, lhsT=kT[0:64, ks], rhs=qT[0:64, qs], start=True, stop=True
                        )
                        nc.tensor.matmul(
                            sB,
                            lhsT=kT[64:128, ks],
                            rhs=qT[64:128, qs],
                            start=True,
                            stop=True,
                        )
                        pA = ppool.tile([128, 128], f32, tag="pA")
                        pB = ppool.tile([128, 128], f32, tag="pB")
                        mcol = mk_sb[:, b * 16 + kj : b * 16 + kj + 1]
                        nc.scalar.activation(pA, sA, AF.Exp, bias=mcol, scale=0.125)
                        nc.scalar.activation(pB, sB, AF.Exp, bias=mcol, scale=0.125)
                        if kj == qi:
                            nc.vector.tensor_mul(pA, pA, causal)
                            nc.vector.tensor_mul(pB, pB, causal)
                        nc.tensor.matmul(
                            o_psA,
                            lhsT=pA,
                            rhs=v[:, kj, 0:65],
                            start=(kj == 0),
                            stop=(kj == qi),
                        )
                        nc.tensor.matmul(
                            o_psB,
                            lhsT=pB,
                            rhs=v[:, kj, 65:130],
                            start=(kj == 0),
                            stop=(kj == qi),
                        )
                    rdA = spool.tile([128, 1], f32, tag="rdA")
                    rdB = spool.tile([128, 1], f32, tag="rdB")
                    nc.vector.reciprocal(rdA, o_psA[:, 64:65])
                    nc.vector.reciprocal(rdB, o_psB[:, 64:65])
                    o_sb = spool.tile([128, 128], f32, tag="o_sb")
                    nc.vector.tensor_scalar_mul(o_sb[:, 0:64], o_psA[:, 0:64], rdA)
                    nc.vector.tensor_scalar_mul(o_sb[:, 64:128], o_psB[:, 0:64], rdB)
                    oT_ps = psml.tile([128, 128], f32, tag="sblk")
                    nc.tensor.transpose(oT_ps, o_sb, ident)
                    oT_sb = spool.tile([128, 128], f32, tag="oT_sb")
                    nc.vector.tensor_copy(oT_sb, oT_ps)
                    for ch in range(2):
                        es = slice(ch * 512, (ch + 1) * 512)
                        op_ps = pbig.tile([128, 512], f32, tag="big")
                        nc.tensor.matmul(
                            op_ps, lhsT=oT_sb, rhs=wo_sb[:, es], start=True, stop=True
                        )
                        out_sb = spool.tile([128, 512], f32, tag="out_sb")
                        nc.vector.tensor_copy(out_sb, op_ps)
                        nc.sync.dma_start(
                            out=out_d[b * S + qi * 128 : b * S + (qi + 1) * 128, es],
                            in_=out_sb,
                        )
    _split_multi_waits(nc)
    return nc


def _get_exec():
    if "exec" in _cache:
        return _cache["exec"]
    _ensure_imports()
    _patch_tile_drain()

    import jax
    import jax.core
    from jax.experimental.shard_map import shard_map
    from jax.sharding import Mesh, PartitionSpec

    import concourse.mybir as mybir
    from concourse import bass2jax

    nc = _build_program()
    bass2jax.install_neuronx_cc_hook()
    assert nc.dbg_addr is None
    partition_name = nc.partition_id_tensor.name if nc.partition_id_tensor else None

    in_names, out_names, out_avals = [], [], []
    for alloc in nc.m.functions[0].allocations:
        if not isinstance(alloc, mybir.MemoryLocationSet):
            continue
        name = alloc.memorylocations[0].name
        if alloc.kind == "ExternalInput":
            if name != partition_name:
                in_names.append(name)
        elif alloc.kind == "ExternalOutput":
            out_names.append(name)
            out_avals.append(
                jax.core.ShapedArray(
                    tuple(alloc.tensor_shape), mybir.dt.np(alloc.dtype)
                )
            )
    n_params = len(in_names)
    all_names = in_names + out_names
    if partition_name is not None:
        all_names = all_names + [partition_name]

    def _body(*args):
        operands = list(args)
        if partition_name is not None:
            operands.append(bass2jax.partition_id_tensor())
        outs = bass2jax._bass_exec_p.bind(
            *operands,
            out_avals=tuple(out_avals),
            in_names=tuple(all_names),
            out_names=tuple(out_names),
            lowering_input_output_aliases=(),
            sim_require_finite=True,
            sim_require_nnan=True,
            nc=nc,
        )
        return tuple(outs)

    devices = jax.devices()[:NCORES]
    assert len(devices) == NCORES, f"need {NCORES} devices, got {len(jax.devices())}"
    mesh = Mesh(np.asarray(devices), ("core",))
    n_outs = len(out_names)
    sharded = jax.jit(
        shard_map(
            _body,
            mesh=mesh,
            in_specs=(PartitionSpec("core"),) * (n_params + n_outs),
            out_specs=(PartitionSpec("core"),) * n_outs,
            check_rep=False,
        ),
        donate_argnums=tuple(range(n_params, n_params + n_outs)),
        keep_unused=True,
    )

    def run(in_maps):
        concat_in = [
            np.concatenate([np.asarray(m[name]) for m in in_maps], axis=0)
            for name in in_names
        ]
        concat_zeros = [
            np.zeros((NCORES * a.shape[0], *a.shape[1:]), a.dtype) for a in out_avals
        ]
        out_arrs = sharded(*concat_in, *concat_zeros)
        return [
            {
                name: np.asarray(out_arrs[i]).reshape(NCORES, *out_avals[i].shape)[c]
                for i, name in enumerate(out_names)
            }
            for c in range(NCORES)
        ]

    _cache["exec"] = run
    return run


def _prep_inputs(x, mask, Wq, bq, Wk, bk, Wv, bv, Wo, bo):
    x = np.asarray(x, np.float32)
    mask = np.asarray(mask)
    xT = np.ascontiguousarray(x.reshape(T, E).T.reshape(8, 128, T))
    maskb = np.where(mask, np.float32(NEG), np.float32(0.0)).astype(np.float32)
    mb = np.ascontiguousarray(maskb.reshape(B, 16, 128).transpose(2, 0, 1).reshape(128, B * 16))
    in_maps = []
    for c in range(NCORES):
        fs = slice(c * FLOC, (c + 1) * FLOC)
        in_maps.append(
            {
                "xT": xT,
                "wqT": np.ascontiguousarray(
                    np.asarray(Wq, np.float32)[fs, :].T.reshape(8, 128, 128)
                ),
                "wkT": np.ascontiguousarray(
                    np.asarray(Wk, np.float32)[fs, :].T.reshape(8, 128, 128)
                ),
                "wvT": np.ascontiguousarray(
                    np.asarray(Wv, np.float32)[fs, :].T.reshape(8, 128, 128)
                ),
                "woT": np.ascontiguousarray(np.asarray(Wo, np.float32)[:, fs].T),
                "bq": np.ascontiguousarray(
                    np.asarray(bq, np.float32)[fs].reshape(128, 1)
                ),
                "maskb": mb,
            }
        )
    return in_maps


def kernel(x, mask, Wq, bq, Wk, bk, Wv, bv, Wo, bo):
    run = _get_exec()
    in_maps = _prep_inputs(x, mask, Wq, bq, Wk, bk, Wv, bv, Wo, bo)
    results = run(in_maps)
    acc = np.zeros((T, E), np.float64)
    for r in results:
        acc += r["out"]
    const = np.asarray(bv, np.float64) @ np.asarray(Wo, np.float64).T + np.asarray(
        bo, np.float64
    )
    out = (acc + const[None, :]).astype(np.float32)
    return out.reshape(B, S, E)


# revision 10
# speedup vs baseline: 2.1470x; 2.1470x over previous
"""Masked multi-head attention (B=2, S=2048, E=1024, H=16) on 8 trn2 cores.

Sharding: tensor-parallel over heads. Each core owns 2 heads (a 128-wide
feature slice of the Q/K/V projections and of Wo's input side), computes the
full projections + causal attention + its partial of the output projection
for all tokens, and writes a full-shape partial output. The host sums the 8
partials and adds the bias terms.

Numerics: all matmuls run as float32r (tf32-class multiplies, ~1.3e-4 rel
for K=1024 dots, 4x the throughput of true fp32 on the PE). Inputs are
declared float32r in DRAM (the PE rounds internally; the host passes raw f32
bits), PSUM accumulation stays fp32, softmax runs in fp32 on ACT/DVE.

Device-side layout avoids all large transposes:
  - qT/kT/vT in [feature, token] layout straight from the projection matmul
    (v is then turned into [token, feature] via 16 128x128 PE transposes)
  - scores computed transposed, sT[k, q] = kT_blk.T @ qT_blk
  - softmax without max-subtraction (scores are O(+-8) here; exp is safe in
    f32) with the denominator produced for free by appending a ones column
    to V in the o = p.T @ [V|1] matmul
  - exp batched over groups of 4 key-blocks to amortize ACT fixed cost;
    the f32->f32r rounding copy of p runs batched on DVE
  - one 128x128 PE transpose per q-tile (o -> oT) feeds the out-projection

Bias/mask handling: bk provably cancels in softmax (constant per row); bv
and bo commute with the attention average (attn rows sum to 1) so both fold
into a host-side constant vector bv @ Wo.T + bo added during the unshard.
The padding mask is multiplicative on exp(scores); when mask is all-False
(the common case) a variant without the mask pass is compiled instead.
"""

import numpy as np

B, S, E, H, D = 2, 2048, 1024, 16, 64
NCORES = 8
FLOC = (H // NCORES) * D  # 128 features per core (2 heads)
T = B * S  # 4096 tokens
NQT = S // 128  # 16 q/k tiles per batch
GRP = 4  # key-blocks per exp batch

_cache = {}


def _ensure_imports():
    try:
        import concourse.bass  # noqa: F401
    except ImportError:
        import sys

        sys.path.append("/opt/trn_rl_repo")


def _patch_tile_drain():
    """walrus CoreV2/V3 codegen accepts only one sync-wait per CTRL
    instruction; TileContext's tail drain carries one per active proc.
    Split them across single-wait nops."""
    import concourse.mybir as mybir
    import concourse.tile as tile_mod
    from concourse.vector_clock import ScopedClock

    if getattr(tile_mod.TileContext, "_drain_patched", False):
        return

    def _drain_and_barrier(self, tick_clock, wait_clock):
        nc = self.nc
        probe = nc.sync.nop(nofuse=True, hint="tail_wait_probe")
        wait_clock.add_sem_waits(
            probe.ins, ScopedClock({None: tick_clock.global_clock})
        )
        waits = list(probe.ins.sync_info.on_wait)
        del probe.ins.sync_info.on_wait[:]
        probe.ins.sync_info.on_wait.extend(waits[:1])
        for i, w in enumerate(waits[1:]):
            carrier = nc.sync.nop(nofuse=True, hint=f"tail_wait_{i}")
            if carrier.ins.sync_info is None:
                carrier.ins.sync_info = mybir.SyncInfo(on_wait=[w], on_update=[])
            else:
                carrier.ins.sync_info.on_wait.append(w)
        nc.sync.drain()
        nc.all_engine_barrier()
        assert self.sems is not None
        popped = nc._tile_sem_poison_stack.pop()
        assert popped is self._sem_poison
        nc.clear_and_free_semaphores(list(self.sems.allocated().values()))
        nc.all_engine_barrier()

    tile_mod.TileContext._drain_and_barrier = _drain_and_barrier
    tile_mod.TileContext._drain_patched = True


def _split_multi_waits(nc):
    """This walrus build accepts only one sync-wait per instruction on
    several instruction encodings (CTRL, LDWEIGHTS). Move extra waits onto
    same-engine nop carriers placed right before the instruction —
    semantically identical (engine queues are in-order)."""
    import concourse.mybir as mybir

    for bb in nc.m.functions[0].blocks:
        new_insts = []
        changed = False
        for inst in bb.instructions:
            si = getattr(inst, "sync_info", None)
            if si is not None and len(si.on_wait) > 1:
                waits = list(si.on_wait)
                del si.on_wait[:]
                si.on_wait.append(waits[-1])
                for i, w in enumerate(waits[:-1]):
                    nop = mybir.InstNoOp(
                        name=f"{inst.name}-sw{i}",
                        engine=inst.engine,
                        sync_info=mybir.SyncInfo(on_wait=[w], on_update=[]),
                        bass_nofuse=True,
                    )
                    nc.register_instruction(nop, overwrite=True)
                    new_insts.append(nop)
                changed = True
            new_insts.append(inst)
        if changed:
            bb.instructions[:] = new_insts


def _build_program(use_mask):
    from contextlib import ExitStack

    import concourse.bass as bass
    import concourse.mybir as mybir
    import concourse.tile as tile
    from concourse.masks import make_identity

    f32 = mybir.dt.float32
    f32r = mybir.dt.float32r
    AF = mybir.ActivationFunctionType

    nc = bass.Bass()
    xT_d = nc.declare_dram_parameter("xT", [8, 128, T], f32r, isOutput=False)
    wq_d = nc.declare_dram_parameter("wqT", [8, 128, 128], f32r, isOutput=False)
    wk_d = nc.declare_dram_parameter("wkT", [8, 128, 128], f32r, isOutput=False)
    wv_d = nc.declare_dram_parameter("wvT", [8, 128, 128], f32r, isOutput=False)
    wo_d = nc.declare_dram_parameter("woT", [128, E], f32r, isOutput=False)
    bq_d = nc.declare_dram_parameter("bq", [128, 1], f32, isOutput=False)
    mk_d = nc.declare_dram_parameter("maskm", [128, B * NQT], f32, isOutput=False)
    out_d = nc.declare_dram_parameter("out", [T, E], f32, isOutput=True)

    with tile.TileContext(nc) as tc:
        with ExitStack() as ctx:
            consts = ctx.enter_context(tc.tile_pool(name="consts", bufs=1))
            xpool = ctx.enter_context(tc.tile_pool(name="xpool", bufs=2))
            qkv = ctx.enter_context(tc.tile_pool(name="qkv", bufs=2))
            ppool = ctx.enter_context(tc.tile_pool(name="ppool", bufs=2))
            spool = ctx.enter_context(tc.tile_pool(name="spool", bufs=2))
            # PSUM budget (8 banks): big=3, sA=1, sB=1, oA+oB=2, tp=1
            pbig = ctx.enter_context(tc.tile_pool(name="pbig", bufs=3, space="PSUM"))
            psml = ctx.enter_context(tc.tile_pool(name="psml", bufs=1, space="PSUM"))
            pso = ctx.enter_context(tc.tile_pool(name="pso", bufs=1, space="PSUM"))
            ptp = ctx.enter_context(tc.tile_pool(name="ptp", bufs=1, space="PSUM"))

            wq_sb = consts.tile([128, 8, 128], f32r)
            wk_sb = consts.tile([128, 8, 128], f32r)
            wv_sb = consts.tile([128, 8, 128], f32r)
            wo_sb = consts.tile([128, E], f32r)
            bq_sb = consts.tile([128, 1], f32)
            mk_sb = consts.tile([128, B * NQT], f32)
            nc.sync.dma_start(out=wq_sb, in_=wq_d[:].rearrange("e p f -> p e f"))
            nc.sync.dma_start(out=wk_sb, in_=wk_d[:].rearrange("e p f -> p e f"))
            nc.sync.dma_start(out=wv_sb, in_=wv_d[:].rearrange("e p f -> p e f"))
            nc.sync.dma_start(out=wo_sb, in_=wo_d[:])
            nc.sync.dma_start(out=bq_sb, in_=bq_d[:])
            if use_mask:
                nc.sync.dma_start(out=mk_sb, in_=mk_d[:])

            ident = consts.tile([128, 128], f32)
            make_identity(nc, ident)

            for b in range(B):
                # ---- projections: qT/kT/vT [feat, tok] then v -> [tok, feat|1]
                qT = qkv.tile([128, S], f32r, tag="qT")
                kT = qkv.tile([128, S], f32r, tag="kT")
                vT = qkv.tile([128, S], f32, tag="vT")
                v = qkv.tile([128, NQT, 132], f32r, tag="v")
                # fp32r matmul needs an even moving free size: [vA|1|0] [vB|1|0]
                nc.gpsimd.memset(v[:, :, 64:66].bitcast(f32), 0.0)
                nc.gpsimd.memset(v[:, :, 130:132].bitcast(f32), 0.0)
                nc.gpsimd.memset(v[:, :, 64:65].bitcast(f32), 1.0)
                nc.gpsimd.memset(v[:, :, 130:131].bitcast(f32), 1.0)
                for tc4 in range(4):
                    g0 = b * S + tc4 * 512
                    xt = xpool.tile([128, 8, 512], f32r, tag="xT")
                    nc.sync.dma_start(
                        out=xt,
                        in_=xT_d[:, :, g0 : g0 + 512].rearrange("e p t -> p e t"),
                    )
                    q_ps = pbig.tile([128, 512], f32, tag="big")
                    k_ps = pbig.tile([128, 512], f32, tag="big")
                    vt_ps = pbig.tile([128, 512], f32, tag="big")
                    for e in range(8):
                        st, sp = (e == 0), (e == 7)
                        nc.tensor.matmul(
                            q_ps, lhsT=wq_sb[:, e, :], rhs=xt[:, e, :], start=st, stop=sp
                        )
                        nc.tensor.matmul(
                            k_ps, lhsT=wk_sb[:, e, :], rhs=xt[:, e, :], start=st, stop=sp
                        )
                        nc.tensor.matmul(
                            vt_ps, lhsT=wv_sb[:, e, :], rhs=xt[:, e, :], start=st, stop=sp
                        )
                    cs = slice(tc4 * 512, (tc4 + 1) * 512)
                    nc.vector.tensor_scalar_add(qT[:, cs], q_ps, bq_sb)
                    nc.vector.tensor_copy(kT[:, cs], k_ps)
                    nc.vector.tensor_copy(vT[:, cs], vt_ps)
                for ti in range(NQT):
                    vt2 = ptp.tile([128, 128], f32, tag="tp")
                    nc.tensor.transpose(
                        vt2, vT[:, ti * 128 : (ti + 1) * 128], ident
                    )
                    nc.vector.tensor_copy(v[:, ti, 0:64], vt2[:, 0:64])
                    nc.vector.tensor_copy(v[:, ti, 66:130], vt2[:, 64:128])

                # ---- causal attention + out-projection, per 128-row q tile
                for qi in range(NQT):
                    qs = slice(qi * 128, (qi + 1) * 128)
                    o_psA = pso.tile([128, 66], f32, tag="oA")
                    o_psB = pso.tile([128, 66], f32, tag="oB")
                    ngrp = (qi + 1 + GRP - 1) // GRP
                    for g in range(ngrp):
                        k0 = g * GRP
                        w = min(GRP, qi + 1 - k0)
                        sA = psml.tile([128, GRP, 128], f32, tag="sA")
                        sB = psml.tile([128, GRP, 128], f32, tag="sB")
                        for r in range(w):
                            kj = k0 + r
                            ks = slice(kj * 128, (kj + 1) * 128)
                            nc.tensor.matmul(
                                sA[:, r, :],
                                lhsT=kT[0:64, ks],
                                rhs=qT[0:64, qs],
                                start=True,
                                stop=True,
                            )
                            nc.tensor.matmul(
                                sB[:, r, :],
                                lhsT=kT[64:128, ks],
                                rhs=qT[64:128, qs],
                                start=True,
                                stop=True,
                            )
                        pa = ppool.tile([128, GRP, 128], f32, tag="pA")
                        pb = ppool.tile([128, GRP, 128], f32, tag="pB")
                        nc.scalar.activation(
                            pa[:, 0:w, :], sA[:, 0:w, :], AF.Exp, bias=0.0, scale=0.125
                        )
                        nc.scalar.activation(
                            pb[:, 0:w, :], sB[:, 0:w, :], AF.Exp, bias=0.0, scale=0.125
                        )
                        if k0 + w == qi + 1:  # group containing the diagonal
                            r = qi - k0
                            for p_ in (pa, pb):
                                # zero strictly-above-diagonal: keep q >= k
                                nc.gpsimd.affine_select(
                                    out=p_[:, r, :],
                                    in_=p_[:, r, :],
                                    compare_op=mybir.AluOpType.is_ge,
                                    fill=0.0,
                                    base=0,
                                    pattern=[[1, 128]],
                                    channel_multiplier=-1,
                                )
                        pa_r = ppool.tile([128, GRP, 128], f32r, tag="pAr")
                        pb_r = ppool.tile([128, GRP, 128], f32r, tag="pBr")
                        if use_mask:
                            for r in range(w):
                                kj = k0 + r
                                mcol = mk_sb[:, b * NQT + kj : b * NQT + kj + 1]
                                nc.vector.tensor_scalar_mul(
                                    pa_r[:, r, :], pa[:, r, :], mcol
                                )
                                nc.vector.tensor_scalar_mul(
                                    pb_r[:, r, :], pb[:, r, :], mcol
                                )
                        else:
                            nc.vector.tensor_copy(pa_r[:, 0:w, :], pa[:, 0:w, :])
                            nc.vector.tensor_copy(pb_r[:, 0:w, :], pb[:, 0:w, :])
                        for r in range(w):
                            kj = k0 + r
                            nc.tensor.matmul(
                                o_psA,
                                lhsT=pa_r[:, r, :],
                                rhs=v[:, kj, 0:66],
                                start=(kj == 0),
                                stop=(kj == qi),
                            )
                            nc.tensor.matmul(
                                o_psB,
                                lhsT=pb_r[:, r, :],
                                rhs=v[:, kj, 66:132],
                                start=(kj == 0),
                                stop=(kj == qi),
                            )
                    rdA = spool.tile([128, 1], f32, tag="rdA")
                    rdB = spool.tile([128, 1], f32, tag="rdB")
                    nc.vector.reciprocal(rdA, o_psA[:, 64:65])
                    nc.vector.reciprocal(rdB, o_psB[:, 64:65])
                    o_sb = spool.tile([128, 128], f32, tag="o_sb")
                    nc.vector.tensor_scalar_mul(o_sb[:, 0:64], o_psA[:, 0:64], rdA)
                    nc.vector.tensor_scalar_mul(o_sb[:, 64:128], o_psB[:, 0:64], rdB)
                    oT_ps = ptp.tile([128, 128], f32, tag="tp")
                    nc.tensor.transpose(oT_ps, o_sb, ident)
                    oT_sb = spool.tile([128, 128], f32r, tag="oT_sb")
                    nc.vector.tensor_copy(oT_sb, oT_ps)
                    for ch in range(2):
                        es = slice(ch * 512, (ch + 1) * 512)
                        op_ps = pbig.tile([128, 512], f32, tag="big")
                        nc.tensor.matmul(
                            op_ps, lhsT=oT_sb, rhs=wo_sb[:, es], start=True, stop=True
                        )
                        out_sb = spool.tile([128, 512], f32, tag="out_sb")
                        nc.vector.tensor_copy(out_sb, op_ps)
                        nc.sync.dma_start(
                            out=out_d[b * S + qi * 128 : b * S + (qi + 1) * 128, es],
                            in_=out_sb,
                        )
    _split_multi_waits(nc)
    return nc


def _get_exec(use_mask):
    key = ("exec", use_mask)
    if key in _cache:
        return _cache[key]
    _ensure_imports()
    _patch_tile_drain()

    import jax
    import jax.core
    from jax.experimental.shard_map import shard_map
    from jax.sharding import Mesh, PartitionSpec

    import concourse.mybir as mybir
    from concourse import bass2jax

    nc = _build_program(use_mask)
    bass2jax.install_neuronx_cc_hook()
    assert nc.dbg_addr is None
    partition_name = nc.partition_id_tensor.name if nc.partition_id_tensor else None

    in_names, out_names, out_avals = [], [], []
    for alloc in nc.m.functions[0].allocations:
        if not isinstance(alloc, mybir.MemoryLocationSet):
            continue
        name = alloc.memorylocations[0].name
        if alloc.kind == "ExternalInput":
            if name != partition_name:
                in_names.append(name)
        elif alloc.kind == "ExternalOutput":
            out_names.append(name)
            out_avals.append(
                jax.core.ShapedArray(
                    tuple(alloc.tensor_shape), mybir.dt.np(alloc.dtype)
                )
            )
    n_params = len(in_names)
    all_names = in_names + out_names
    if partition_name is not None:
        all_names = all_names + [partition_name]

    def _body(*args):
        operands = list(args)
        if partition_name is not None:
            operands.append(bass2jax.partition_id_tensor())
        outs = bass2jax._bass_exec_p.bind(
            *operands,
            out_avals=tuple(out_avals),
            in_names=tuple(all_names),
            out_names=tuple(out_names),
            lowering_input_output_aliases=(),
            sim_require_finite=True,
            sim_require_nnan=True,
            nc=nc,
        )
        return tuple(outs)

    devices = jax.devices()[:NCORES]
    assert len(devices) == NCORES, f"need {NCORES} devices, got {len(jax.devices())}"
    mesh = Mesh(np.asarray(devices), ("core",))
    n_outs = len(out_names)
    sharded = jax.jit(
        shard_map(
            _body,
            mesh=mesh,
            in_specs=(PartitionSpec("core"),) * (n_params + n_outs),
            out_specs=(PartitionSpec("core"),) * n_outs,
            check_rep=False,
        ),
        donate_argnums=tuple(range(n_params, n_params + n_outs)),
        keep_unused=True,
    )

    def run(in_maps):
        concat_in = [
            np.concatenate([np.asarray(m[name]) for m in in_maps], axis=0)
            for name in in_names
        ]
        concat_zeros = [
            np.zeros((NCORES * a.shape[0], *a.shape[1:]), a.dtype) for a in out_avals
        ]
        out_arrs = sharded(*concat_in, *concat_zeros)
        return [
            {
                name: np.asarray(out_arrs[i]).reshape(NCORES, *out_avals[i].shape)[c]
                for i, name in enumerate(out_names)
            }
            for c in range(NCORES)
        ]

    _cache[key] = run
    return run


def _prep_inputs(x, mask, Wq, bq, Wk, bk, Wv, bv, Wo, bo):
    x = np.asarray(x, np.float32)
    mask = np.asarray(mask)
    xT = np.ascontiguousarray(x.reshape(T, E).T.reshape(8, 128, T))
    # multiplicative key mask, [part, b*16+kj] layout
    maskm = np.where(mask, np.float32(0.0), np.float32(1.0))
    mm = np.ascontiguousarray(
        maskm.reshape(B, NQT, 128).transpose(2, 0, 1).reshape(128, B * NQT)
    )
    in_maps = []
    for c in range(NCORES):
        fs = slice(c * FLOC, (c + 1) * FLOC)
        in_maps.append(
            {
                "xT": xT,
                "wqT": np.ascontiguousarray(
                    np.asarray(Wq, np.float32)[fs, :].T.reshape(8, 128, 128)
                ),
                "wkT": np.ascontiguousarray(
                    np.asarray(Wk, np.float32)[fs, :].T.reshape(8, 128, 128)
                ),
                "wvT": np.ascontiguousarray(
                    np.asarray(Wv, np.float32)[fs, :].T.reshape(8, 128, 128)
                ),
                "woT": np.ascontiguousarray(np.asarray(Wo, np.float32)[:, fs].T),
                "bq": np.ascontiguousarray(
                    np.asarray(bq, np.float32)[fs].reshape(128, 1)
                ),
                "maskm": mm,
            }
        )
    return in_maps


def kernel(x, mask, Wq, bq, Wk, bk, Wv, bv, Wo, bo):
    use_mask = bool(np.asarray(mask).any())
    run = _get_exec(use_mask)
    in_maps = _prep_inputs(x, mask, Wq, bq, Wk, bk, Wv, bv, Wo, bo)
    results = run(in_maps)
    acc = np.zeros((T, E), np.float64)
    for r in results:
        acc += r["out"]
    const = np.asarray(bv, np.float64) @ np.asarray(Wo, np.float64).T + np.asarray(
        bo, np.float64
    )
    out = (acc + const[None, :]).astype(np.float32)
    return out.reshape(B, S, E)
